# revision 7
# baseline (speedup 1.0000x reference)
"""Trainium2 Bass kernel for nn_LinearCoeffGNN: coeffs = U @ Vp^T pipeline.

Math (exact factorization of the reference):
  Linear(1,hid) layers make Q/K/V rank-1 in x, so the kernelized-attention
  block collapses: scores softmax needs only exp(x_p * A[h,m]) column stats,
  mem_KV is rank-1, and h = alpha*w_v + beta*b_v with (alpha,beta) linear in
  (qv,qb) via per-head scalars S1=sum_m s, S2=sum_m s^2.  Final output is
  coeffs[b] = F0 @ N @ F0^T with F0 = [qv_h | qb_h | 1] (P x 17) and
  N = T' M' T' (17x17, data-dependent via S1/S2 only).
Sharding: data-parallel over batch B=32 -> 4 batches per core on 8 cores.
"""
import numpy as np

import concourse.bacc as bacc
import concourse.bass as bass
import concourse.mybir as mybir
import concourse.tile as tile
from concourse import bass_utils

B, P = 32, 1024
HID, H, D = 512, 8, 64
MEM, RANK = 64, 64
NCORES = 8
BPC = B // NCORES  # batches per core
HM = H * MEM  # 512

F32 = mybir.dt.float32
F32R = mybir.dt.float32r
BF16 = mybir.dt.bfloat16
AF = mybir.ActivationFunctionType
ALU = mybir.AluOpType

_CACHE = {}
TRACE = False


def _build():
    nc = bacc.Bacc("TRN2", target_bir_lowering=False, debug=False,
                   num_devices=NCORES)
    xs = nc.dram_tensor("xs", [BPC, P], F32, kind="ExternalInput").ap()
    xo = nc.dram_tensor("xo", [BPC, 128, 16], F32, kind="ExternalInput").ap()
    abc = nc.dram_tensor("abc", [1, HM], F32, kind="ExternalInput").ap()
    wqbq = nc.dram_tensor("wqbq", [128, 8], F32, kind="ExternalInput").ap()
    wdd = nc.dram_tensor("wdd", [128, 64], F32, kind="ExternalInput").ap()
    maskA = nc.dram_tensor("maskA", [17, 17], F32, kind="ExternalInput").ap()
    maskB = nc.dram_tensor("maskB", [17, 17], F32, kind="ExternalInput").ap()
    constT = nc.dram_tensor("constT", [17, 17], F32, kind="ExternalInput").ap()
    mT = nc.dram_tensor("mT", [17, 17], F32, kind="ExternalInput").ap()
    permAB = nc.dram_tensor("permAB", [16, 49], F32, kind="ExternalInput").ap()
    ident = nc.dram_tensor("ident", [1, 1], F32, kind="ExternalInput").ap()
    out = nc.dram_tensor("out", [BPC, P, P], F32, kind="ExternalOutput").ap()

    with tile.TileContext(nc) as tc:
        with tc.tile_pool(name="consts", bufs=1) as cp, \
             tc.tile_pool(name="work", bufs=2) as wp, \
             tc.tile_pool(name="ework", bufs=3) as ep, \
             tc.tile_pool(name="stage", bufs=3) as sp, \
             tc.tile_pool(name="ps_small", bufs=2, space="PSUM") as pss, \
             tc.tile_pool(name="ps_f", bufs=1, space="PSUM") as psf, \
             tc.tile_pool(name="ps_z", bufs=1, space="PSUM") as psz, \
             tc.tile_pool(name="ps_cc", bufs=2, space="PSUM") as psc:

            # ---- constants (loaded once) ----
            a_bc = cp.tile([128, HM], F32, tag="a_bc")
            nc.sync.dma_start(out=a_bc, in_=bass.AP(
                tensor=abc.tensor, offset=abc.offset,
                ap=[[0, 128]] + abc.ap[1:]))
            wqbq_sb = cp.tile([128, 8], F32, tag="wqbq")
            nc.sync.dma_start(out=wqbq_sb, in_=wqbq)
            wd_sb = cp.tile([128, 64], F32, tag="wd_sb")
            nc.sync.dma_start(out=wd_sb, in_=wdd)
            wd_r = cp.tile([128, 64], BF16, tag="wd_r")
            nc.vector.tensor_copy(wd_r, wd_sb)
            mA_sb = cp.tile([17, 17], F32, tag="mA")
            nc.sync.dma_start(out=mA_sb, in_=maskA)
            mB_sb = cp.tile([17, 17], F32, tag="mB")
            nc.sync.dma_start(out=mB_sb, in_=maskB)
            cT_sb = cp.tile([17, 17], F32, tag="cT")
            nc.sync.dma_start(out=cT_sb, in_=constT)
            mT_sb = cp.tile([17, 17], F32, tag="mT")
            nc.sync.dma_start(out=mT_sb, in_=mT)
            perm_sb = cp.tile([16, 49], F32, tag="perm")
            nc.sync.dma_start(out=perm_sb, in_=permAB)
            id_sb = cp.tile([1, 1], F32, tag="ident")
            nc.sync.dma_start(out=id_sb, in_=ident)
            ones_f = cp.tile([1, P], F32, tag="ones_f")
            nc.vector.memset(ones_f, 1.0)
            ones_row = cp.tile([1, P], BF16, tag="ones_row")
            nc.vector.tensor_copy(ones_row, ones_f)

            for b in range(BPC):
                # x broadcast to all partitions (x along free dim)
                xrow = xs[b, :]
                xb_sb = wp.tile([128, P], F32, tag="xb")
                nc.sync.dma_start(out=xb_sb, in_=bass.AP(
                    tensor=xrow.tensor, offset=xrow.offset,
                    ap=[[0, 128]] + xrow.ap))
                xo_sb = wp.tile([128, 16], F32, tag="xo")
                nc.sync.dma_start(out=xo_sb, in_=xo[b])
                xo_r = wp.tile([128, 16], BF16, tag="xor")
                nc.vector.tensor_copy(xo_r, xo_sb)

                # ---- phi = min(exp(u),1) + relu(u), u = x*wq + bq ----
                # layout [hid_chunk(128part), p(1024)]
                fts = []
                for c in range(4):
                    e_c = wp.tile([128, P], F32, tag="e")
                    nc.scalar.activation(e_c, xb_sb, AF.Exp,
                                         bias=wqbq_sb[:, 4 + c:5 + c],
                                         scale=wqbq_sb[:, c:c + 1])
                    u_c = wp.tile([128, P], F32, tag="u")
                    nc.gpsimd.tensor_scalar(u_c, xb_sb,
                                            wqbq_sb[:, c:c + 1],
                                            wqbq_sb[:, 4 + c:5 + c],
                                            op0=ALU.mult, op1=ALU.add)
                    r_c = wp.tile([128, P], F32, tag="r")
                    nc.gpsimd.tensor_scalar_max(r_c, u_c, 0.0)
                    ft_c = wp.tile([128, P], BF16, tag=f"ft{c}")
                    nc.vector.scalar_tensor_tensor(
                        ft_c, e_c, 1.0, r_c, op0=ALU.min, op1=ALU.add)
                    fts.append(ft_c)

                # qv/qb: f_ps[j, p] = sum_hid Wd[hid,j] * phi[hid, p]
                f_ps = psf.tile([16, P], F32, tag="fps")
                for half in range(2):
                    for c in range(4):
                        nc.tensor.matmul(
                            f_ps[:, half * 512:(half + 1) * 512],
                            wd_r[:, c * 16:(c + 1) * 16],
                            fts[c][:, half * 512:(half + 1) * 512],
                            start=(c == 0), stop=(c == 3))
                f0t = wp.tile([17, P], BF16, tag="f0t")
                nc.vector.tensor_copy(f0t[0:16, :], f_ps)
                nc.sync.dma_start(out=f0t[16:17, :], in_=ones_row)

                # ---- softmax stats: E = exp(x_p * A[hm]) ----
                # layout [p_chunk(128part), hm(512)]
                num_ps = pss.tile([1, 512], F32, tag="small")
                den_ps = pss.tile([1, 512], F32, tag="small")
                for c in range(8):
                    e2_c = ep.tile([128, HM], BF16, tag="E")
                    nc.scalar.activation(e2_c, a_bc, AF.Exp,
                                         scale=xo_sb[:, 2 * c:2 * c + 1])
                    nc.tensor.matmul(num_ps, xo_r[:, 2 * c:2 * c + 1], e2_c,
                                     start=(c == 0), stop=(c == 7))
                    nc.tensor.matmul(den_ps, xo_r[:, 2 * c + 1:2 * c + 2], e2_c,
                                     start=(c == 0), stop=(c == 7))
                rec = wp.tile([1, 512], F32, tag="rec")
                rscr = wp.tile([1, 512], F32, tag="rscr")
                nc.vector.reciprocal_approx_accurate(rec, den_ps[0:1, :],
                                                     scratch=rscr)
                s_sb = wp.tile([1, 512], F32, tag="s")
                nc.vector.tensor_mul(s_sb, num_ps[0:1, :], rec)
                s2_sb = wp.tile([1, 512], F32, tag="s2")
                nc.vector.tensor_mul(s2_sb, s_sb, s_sb)
                sred = wp.tile([1, 16], F32, tag="sred")
                nc.vector.reduce_sum(sred[0:1, 0:8],
                                     s_sb.rearrange("a (h m) -> a h m", h=8),
                                     axis=mybir.AxisListType.X)
                nc.vector.reduce_sum(sred[0:1, 8:16],
                                     s2_sb.rearrange("a (h m) -> a h m", h=8),
                                     axis=mybir.AxisListType.X)
                scol_ps = pss.tile([16, 1], F32, tag="small")
                nc.tensor.transpose(scol_ps, sred, id_sb)
                scol_sb = wp.tile([16, 1], F32, tag="scolsb")
                nc.vector.tensor_copy(scol_sb, scol_ps)
                ab_ps = pss.tile([49, 1], F32, tag="small")
                nc.tensor.matmul(ab_ps, perm_sb, scol_sb, start=True, stop=True)

                # T' build + N = T' M' T'
                t1 = wp.tile([17, 17], F32, tag="t1")
                nc.vector.scalar_tensor_tensor(
                    t1, mA_sb, ab_ps[0:17, 0:1], cT_sb,
                    op0=ALU.mult, op1=ALU.add)
                tp_sb = wp.tile([17, 17], F32, tag="tp")
                nc.vector.scalar_tensor_tensor(
                    tp_sb, mB_sb, ab_ps[32:49, 0:1], t1,
                    op0=ALU.mult, op1=ALU.add)
                p1_ps = pss.tile([17, 17], F32, tag="small")
                nc.tensor.matmul(p1_ps, mT_sb, tp_sb, start=True, stop=True)
                p1_sb = wp.tile([17, 17], F32, tag="p1sb")
                nc.vector.tensor_copy(p1_sb, p1_ps)
                n_ps = pss.tile([17, 17], F32, tag="small")
                nc.tensor.matmul(n_ps, tp_sb, p1_sb, start=True, stop=True)
                n_sb = wp.tile([17, 17], BF16, tag="nsb")
                nc.vector.tensor_copy(n_sb, n_ps)

                # Z = N^T @ F0^T  [17, 1024]
                z_ps = psz.tile([17, P], F32, tag="zps")
                for half in range(2):
                    nc.tensor.matmul(z_ps[:, half * 512:(half + 1) * 512],
                                     n_sb, f0t[:, half * 512:(half + 1) * 512],
                                     start=True, stop=True)
                z_sb = wp.tile([17, P], BF16, tag="zsb")
                nc.vector.tensor_copy(z_sb, z_ps)

                # coeffs chunk rows: out[b, rc*128:(rc+1)*128, :] =
                #   Z[:, chunk]^T @ F0^T
                for rc in range(8):
                    st = sp.tile([128, P], F32, tag="st")
                    for half in range(2):
                        cc_ps = psc.tile([128, 512], F32, tag="cc")
                        nc.tensor.matmul(
                            cc_ps, z_sb[:, rc * 128:(rc + 1) * 128],
                            f0t[:, half * 512:(half + 1) * 512],
                            start=True, stop=True)
                        nc.any.tensor_copy(
                            st[:, half * 512:(half + 1) * 512], cc_ps)
                    nc.sync.dma_start(
                        out=out[b, rc * 128:(rc + 1) * 128, :], in_=st)
    nc.compile()
    return nc


def _host_consts(w_q, b_q, w_k, b_k, w_v, b_v, w_mem, w_u, b_u, w_v2, b_v2):
    A = (w_k.reshape(H, D) @ w_mem.T).astype(np.float32)       # (H, MEM)
    Wd = np.zeros((HID, 16), np.float32)
    Gu = np.zeros((17, RANK), np.float32)
    Gv = np.zeros((17, RANK), np.float32)
    for h in range(H):
        sl = slice(h * D, (h + 1) * D)
        Wd[sl, 2 * h] = w_v[sl]
        Wd[sl, 2 * h + 1] = b_v[sl]
        Gu[2 * h] = w_u[:, sl] @ w_v[sl]
        Gu[2 * h + 1] = w_u[:, sl] @ b_v[sl]
        Gv[2 * h] = w_v2[:, sl] @ w_v[sl]
        Gv[2 * h + 1] = w_v2[:, sl] @ b_v[sl]
    Gu[16] = b_u
    Gv[16] = b_v2
    Mp = (Gu @ Gv.T).astype(np.float32)                         # (17,17)
    mA = np.zeros((17, 17), np.float32)
    mB = np.zeros((17, 17), np.float32)
    cT = np.zeros((17, 17), np.float32)
    perm = np.zeros((16, 49), np.float32)
    for h in range(H):
        mA[2 * h, 2 * h] = 1.0
        mB[2 * h, 2 * h + 1] = 1.0
        mB[2 * h + 1, 2 * h] = 1.0
        cT[2 * h + 1, 2 * h + 1] = float(MEM)
        # s_col = [S1_0..S1_7, S2_0..S2_7]; a_vec[2h]=S2_h; b_vec[2h]=b_vec[2h+1]=S1_h
        perm[8 + h, 2 * h] = 1.0
        perm[h, 32 + 2 * h] = 1.0
        perm[h, 32 + 2 * h + 1] = 1.0
    cT[16, 16] = 1.0
    consts = {
        "abc": A.reshape(1, HM),
        "wqbq": np.stack([w_q.reshape(4, 128), b_q.reshape(4, 128)],
                         0).reshape(8, 128).T.copy(),
        "wdd": Wd.reshape(4, 128, 16).transpose(1, 0, 2).reshape(128, 64).copy(),
        "maskA": mA, "maskB": mB, "constT": cT, "mT": Mp.T.copy(),
        "permAB": perm, "ident": np.ones((1, 1), np.float32),
    }
    return consts


def kernel(**inputs):
    x = np.ascontiguousarray(inputs["x"], dtype=np.float32)
    consts = _host_consts(
        *(np.asarray(inputs[k], np.float32) for k in
          ["w_q", "b_q", "w_k", "b_k", "w_v", "b_v", "w_mem",
           "w_u", "b_u", "w_v2", "b_v2"]))
    if "nc" not in _CACHE:
        _CACHE["nc"] = _build()
    nc = _CACHE["nc"]
    in_maps = []
    for c in range(NCORES):
        xs = x[c * BPC:(c + 1) * BPC]                            # (BPC, P)
        # xo: even cols = x chunks (col-major), odd cols = ones
        xo = np.ones((BPC, 128, 16), np.float32)
        xo[:, :, 0:16:2] = xs.reshape(BPC, 8, 128).transpose(0, 2, 1)
        in_maps.append({"xs": xs.copy(), "xo": xo, **consts})
    res = bass_utils.run_bass_kernel_spmd(
        nc, in_maps, core_ids=list(range(NCORES)), trace=TRACE)
    _CACHE["last_res"] = res
    return np.concatenate([res.results[c]["out"] for c in range(NCORES)], 0)


# revision 9
# speedup vs baseline: 2.4970x; 2.4970x over previous
"""Trainium2 Bass kernel for nn_LinearCoeffGNN: coeffs = U @ Vp^T pipeline.

Math (exact factorization of the reference):
  Linear(1,hid) layers make Q/K/V rank-1 in x, so the kernelized-attention
  block collapses: scores softmax needs only exp(x_p * A[h,m]) column stats,
  mem_KV is rank-1, and h = alpha*w_v + beta*b_v with (alpha,beta) linear in
  (qv,qb) via per-head scalars S1=sum_m s, S2=sum_m s^2.  Final output is
  coeffs[b] = F0 @ N @ F0^T with F0 = [qv_h | qb_h | 1] (P x 17) and
  N = T' M' T' (17x17, data-dependent via S1/S2 only).
Sharding: data-parallel over batch B=32 -> 4 batches per core on 8 cores.
"""
import numpy as np

import concourse.bacc as bacc
import concourse.bass as bass
import concourse.mybir as mybir
import concourse.tile as tile
from concourse import bass_utils

B, P = 32, 1024
HID, H, D = 512, 8, 64
MEM, RANK = 64, 64
NCORES = 8
BPC = B // NCORES  # batches per core
HM = H * MEM  # 512

F32 = mybir.dt.float32
F32R = mybir.dt.float32r
BF16 = mybir.dt.bfloat16
AF = mybir.ActivationFunctionType
ALU = mybir.AluOpType

_CACHE = {}
TRACE = False


def _build():
    nc = bacc.Bacc("TRN2", target_bir_lowering=False, debug=False,
                   num_devices=NCORES)
    xs = nc.dram_tensor("xs", [BPC, P], F32, kind="ExternalInput").ap()
    xo = nc.dram_tensor("xo", [BPC, 128, 16], F32, kind="ExternalInput").ap()
    abc = nc.dram_tensor("abc", [1, HM], F32, kind="ExternalInput").ap()
    wqbq = nc.dram_tensor("wqbq", [128, 8], F32, kind="ExternalInput").ap()
    wdd = nc.dram_tensor("wdd", [128, 64], F32, kind="ExternalInput").ap()
    maskA = nc.dram_tensor("maskA", [17, 17], F32, kind="ExternalInput").ap()
    maskB = nc.dram_tensor("maskB", [17, 17], F32, kind="ExternalInput").ap()
    constT = nc.dram_tensor("constT", [17, 17], F32, kind="ExternalInput").ap()
    mT = nc.dram_tensor("mT", [17, 17], F32, kind="ExternalInput").ap()
    permAB = nc.dram_tensor("permAB", [16, 49], F32, kind="ExternalInput").ap()
    ident = nc.dram_tensor("ident", [1, 1], F32, kind="ExternalInput").ap()
    out = nc.dram_tensor("out", [BPC, P, P], F32, kind="ExternalOutput").ap()

    with tile.TileContext(nc) as tc:
        with tc.tile_pool(name="consts", bufs=1) as cp, \
             tc.tile_pool(name="work", bufs=2) as wp, \
             tc.tile_pool(name="ework", bufs=3) as ep, \
             tc.tile_pool(name="stage", bufs=3) as sp, \
             tc.tile_pool(name="ps_small", bufs=2, space="PSUM") as pss, \
             tc.tile_pool(name="ps_f", bufs=1, space="PSUM") as psf, \
             tc.tile_pool(name="ps_z", bufs=1, space="PSUM") as psz, \
             tc.tile_pool(name="ps_cc", bufs=2, space="PSUM") as psc:

            # ---- constants (loaded once) ----
            a_bc = cp.tile([128, HM], F32, tag="a_bc")
            nc.sync.dma_start(out=a_bc, in_=bass.AP(
                tensor=abc.tensor, offset=abc.offset,
                ap=[[0, 128]] + abc.ap[1:]))
            wqbq_sb = cp.tile([128, 8], F32, tag="wqbq")
            nc.sync.dma_start(out=wqbq_sb, in_=wqbq)
            wd_sb = cp.tile([128, 64], F32, tag="wd_sb")
            nc.sync.dma_start(out=wd_sb, in_=wdd)
            wd_r = cp.tile([128, 64], F32R, tag="wd_r")
            nc.vector.tensor_copy(wd_r, wd_sb)
            mA_sb = cp.tile([17, 17], F32, tag="mA")
            nc.sync.dma_start(out=mA_sb, in_=maskA)
            mB_sb = cp.tile([17, 17], F32, tag="mB")
            nc.sync.dma_start(out=mB_sb, in_=maskB)
            cT_sb = cp.tile([17, 17], F32, tag="cT")
            nc.sync.dma_start(out=cT_sb, in_=constT)
            mT_sb = cp.tile([17, 17], F32, tag="mT")
            nc.sync.dma_start(out=mT_sb, in_=mT)
            perm_sb = cp.tile([16, 49], F32, tag="perm")
            nc.sync.dma_start(out=perm_sb, in_=permAB)
            id_sb = cp.tile([1, 1], F32, tag="ident")
            nc.sync.dma_start(out=id_sb, in_=ident)
            ones_f = cp.tile([1, P], F32, tag="ones_f")
            nc.vector.memset(ones_f, 1.0)
            ones_row = cp.tile([1, P], F32R, tag="ones_row")
            nc.vector.tensor_copy(ones_row, ones_f)

            for b in range(BPC):
                # x broadcast to all partitions (x along free dim)
                xrow = xs[b, :]
                xb_sb = wp.tile([128, P], F32, tag="xb")
                nc.sync.dma_start(out=xb_sb, in_=bass.AP(
                    tensor=xrow.tensor, offset=xrow.offset,
                    ap=[[0, 128]] + xrow.ap))
                xo_sb = wp.tile([128, 16], F32, tag="xo")
                nc.sync.dma_start(out=xo_sb, in_=xo[b])
                xo_r = wp.tile([128, 16], F32R, tag="xor")
                nc.vector.tensor_copy(xo_r, xo_sb)

                # ---- phi = min(exp(u),1) + relu(u), u = x*wq + bq ----
                # layout [hid_chunk(128part), p(1024)]
                fts = []
                for c in range(4):
                    e_c = wp.tile([128, P], F32, tag="e")
                    nc.scalar.activation(e_c, xb_sb, AF.Exp,
                                         bias=wqbq_sb[:, 4 + c:5 + c],
                                         scale=wqbq_sb[:, c:c + 1])
                    r_c = wp.tile([128, P], F32, tag="r")
                    nc.scalar.activation(r_c, xb_sb, AF.Relu,
                                         bias=wqbq_sb[:, 4 + c:5 + c],
                                         scale=wqbq_sb[:, c:c + 1])
                    ft_c = wp.tile([128, P], F32R, tag=f"ft{c}")
                    nc.vector.scalar_tensor_tensor(
                        ft_c, e_c, 1.0, r_c, op0=ALU.min, op1=ALU.add)
                    fts.append(ft_c)

                # qv/qb: f_ps[j, p] = sum_hid Wd[hid,j] * phi[hid, p]
                f_ps = psf.tile([16, P], F32, tag="fps")
                for half in range(2):
                    for c in range(4):
                        nc.tensor.matmul(
                            f_ps[:, half * 512:(half + 1) * 512],
                            wd_r[:, c * 16:(c + 1) * 16],
                            fts[c][:, half * 512:(half + 1) * 512],
                            start=(c == 0), stop=(c == 3))
                f0t = wp.tile([17, P], F32R, tag="f0t")
                nc.vector.tensor_copy(f0t[0:16, :], f_ps)
                nc.sync.dma_start(out=f0t[16:17, :], in_=ones_row)

                # ---- softmax stats: E = exp(x_p * A[hm]) ----
                # layout [p_chunk(128part), hm(512)]
                num_ps = pss.tile([1, 512], F32, tag="small")
                den_ps = pss.tile([1, 512], F32, tag="small")
                for c in range(8):
                    e2_c = ep.tile([128, HM], F32R, tag="E")
                    nc.scalar.activation(e2_c, a_bc, AF.Exp,
                                         scale=xo_sb[:, 2 * c:2 * c + 1])
                    nc.tensor.matmul(num_ps, xo_r[:, 2 * c:2 * c + 1], e2_c,
                                     start=(c == 0), stop=(c == 7))
                    nc.tensor.matmul(den_ps, xo_r[:, 2 * c + 1:2 * c + 2], e2_c,
                                     start=(c == 0), stop=(c == 7))
                rec = wp.tile([1, 512], F32, tag="rec")
                rscr = wp.tile([1, 512], F32, tag="rscr")
                nc.vector.reciprocal_approx_accurate(rec, den_ps[0:1, :],
                                                     scratch=rscr)
                s_sb = wp.tile([1, 512], F32, tag="s")
                nc.vector.tensor_mul(s_sb, num_ps[0:1, :], rec)
                s2_sb = wp.tile([1, 512], F32, tag="s2")
                nc.vector.tensor_mul(s2_sb, s_sb, s_sb)
                sred = wp.tile([1, 16], F32, tag="sred")
                nc.vector.reduce_sum(sred[0:1, 0:8],
                                     s_sb.rearrange("a (h m) -> a h m", h=8),
                                     axis=mybir.AxisListType.X)
                nc.vector.reduce_sum(sred[0:1, 8:16],
                                     s2_sb.rearrange("a (h m) -> a h m", h=8),
                                     axis=mybir.AxisListType.X)
                scol_ps = pss.tile([16, 1], F32, tag="small")
                nc.tensor.transpose(scol_ps, sred, id_sb)
                scol_sb = wp.tile([16, 1], F32, tag="scolsb")
                nc.vector.tensor_copy(scol_sb, scol_ps)
                ab_ps = pss.tile([49, 1], F32, tag="small")
                nc.tensor.matmul(ab_ps, perm_sb, scol_sb, start=True, stop=True)

                # T' build + N = T' M' T'
                t1 = wp.tile([17, 17], F32, tag="t1")
                nc.vector.scalar_tensor_tensor(
                    t1, mA_sb, ab_ps[0:17, 0:1], cT_sb,
                    op0=ALU.mult, op1=ALU.add)
                tp_sb = wp.tile([17, 17], F32, tag="tp")
                nc.vector.scalar_tensor_tensor(
                    tp_sb, mB_sb, ab_ps[32:49, 0:1], t1,
                    op0=ALU.mult, op1=ALU.add)
                p1_ps = pss.tile([17, 17], F32, tag="small")
                nc.tensor.matmul(p1_ps, mT_sb, tp_sb, start=True, stop=True)
                p1_sb = wp.tile([17, 17], F32, tag="p1sb")
                nc.vector.tensor_copy(p1_sb, p1_ps)
                n_ps = pss.tile([17, 17], F32, tag="small")
                nc.tensor.matmul(n_ps, tp_sb, p1_sb, start=True, stop=True)
                n_sb = wp.tile([17, 17], F32R, tag="nsb")
                nc.vector.tensor_copy(n_sb, n_ps)

                # Z = N^T @ F0^T  [17, 1024]
                z_ps = psz.tile([17, P], F32, tag="zps")
                for half in range(2):
                    nc.tensor.matmul(z_ps[:, half * 512:(half + 1) * 512],
                                     n_sb, f0t[:, half * 512:(half + 1) * 512],
                                     start=True, stop=True)
                z_sb = wp.tile([17, P], F32R, tag="zsb")
                nc.vector.tensor_copy(z_sb, z_ps)

                # coeffs chunk rows: out[b, rc*128:(rc+1)*128, :] =
                #   Z[:, chunk]^T @ F0^T
                for rc in range(8):
                    st = sp.tile([128, P], F32, tag="st")
                    for half in range(2):
                        cc_ps = psc.tile([128, 512], F32, tag="cc")
                        nc.tensor.matmul(
                            cc_ps, z_sb[:, rc * 128:(rc + 1) * 128],
                            f0t[:, half * 512:(half + 1) * 512],
                            start=True, stop=True)
                        nc.any.tensor_copy(
                            st[:, half * 512:(half + 1) * 512], cc_ps)
                    nc.sync.dma_start(
                        out=out[b, rc * 128:(rc + 1) * 128, :], in_=st)
    nc.compile()
    return nc


def _host_consts(w_q, b_q, w_k, b_k, w_v, b_v, w_mem, w_u, b_u, w_v2, b_v2):
    A = (w_k.reshape(H, D) @ w_mem.T).astype(np.float32)       # (H, MEM)
    Wd = np.zeros((HID, 16), np.float32)
    Gu = np.zeros((17, RANK), np.float32)
    Gv = np.zeros((17, RANK), np.float32)
    for h in range(H):
        sl = slice(h * D, (h + 1) * D)
        Wd[sl, 2 * h] = w_v[sl]
        Wd[sl, 2 * h + 1] = b_v[sl]
        Gu[2 * h] = w_u[:, sl] @ w_v[sl]
        Gu[2 * h + 1] = w_u[:, sl] @ b_v[sl]
        Gv[2 * h] = w_v2[:, sl] @ w_v[sl]
        Gv[2 * h + 1] = w_v2[:, sl] @ b_v[sl]
    Gu[16] = b_u
    Gv[16] = b_v2
    Mp = (Gu @ Gv.T).astype(np.float32)                         # (17,17)
    mA = np.zeros((17, 17), np.float32)
    mB = np.zeros((17, 17), np.float32)
    cT = np.zeros((17, 17), np.float32)
    perm = np.zeros((16, 49), np.float32)
    for h in range(H):
        mA[2 * h, 2 * h] = 1.0
        mB[2 * h, 2 * h + 1] = 1.0
        mB[2 * h + 1, 2 * h] = 1.0
        cT[2 * h + 1, 2 * h + 1] = float(MEM)
        # s_col = [S1_0..S1_7, S2_0..S2_7]; a_vec[2h]=S2_h; b_vec[2h]=b_vec[2h+1]=S1_h
        perm[8 + h, 2 * h] = 1.0
        perm[h, 32 + 2 * h] = 1.0
        perm[h, 32 + 2 * h + 1] = 1.0
    cT[16, 16] = 1.0
    consts = {
        "abc": A.reshape(1, HM),
        "wqbq": np.stack([w_q.reshape(4, 128), b_q.reshape(4, 128)],
                         0).reshape(8, 128).T.copy(),
        "wdd": Wd.reshape(4, 128, 16).transpose(1, 0, 2).reshape(128, 64).copy(),
        "maskA": mA, "maskB": mB, "constT": cT, "mT": Mp.T.copy(),
        "permAB": perm, "ident": np.ones((1, 1), np.float32),
    }
    return consts


def kernel(**inputs):
    x = np.ascontiguousarray(inputs["x"], dtype=np.float32)
    consts = _host_consts(
        *(np.asarray(inputs[k], np.float32) for k in
          ["w_q", "b_q", "w_k", "b_k", "w_v", "b_v", "w_mem",
           "w_u", "b_u", "w_v2", "b_v2"]))
    if "nc" not in _CACHE:
        _CACHE["nc"] = _build()
    nc = _CACHE["nc"]
    in_maps = []
    for c in range(NCORES):
        xs = x[c * BPC:(c + 1) * BPC]                            # (BPC, P)
        # xo: even cols = x chunks (col-major), odd cols = ones
        xo = np.ones((BPC, 128, 16), np.float32)
        xo[:, :, 0:16:2] = xs.reshape(BPC, 8, 128).transpose(0, 2, 1)
        in_maps.append({"xs": xs.copy(), "xo": xo, **consts})
    res = bass_utils.run_bass_kernel_spmd(
        nc, in_maps, core_ids=list(range(NCORES)), trace=TRACE)
    _CACHE["last_res"] = res
    return np.concatenate([res.results[c]["out"] for c in range(NCORES)], 0)
